# revision 29
# baseline (speedup 1.0000x reference)
# Trainium2 Bass kernel for nn_Discriminator (dense transformer).
#
# Sharding: pure data-parallel over batch. 256 samples -> 8 NeuronCores x 32.
# All ~30M params are replicated (packed into one fp32 blob + one fp16 blob).
#
# Layout strategy: activations stay resident in SBUF *transposed* -- hidden
# dim on partitions (4 chunks of 128), rows (sample*seq) on the free dim.
# Every linear is then a weight-stationary PE matmul in float32r (full rate
# at N>=256); biases and activations fuse into the PSUM->SBUF eviction on
# the scalar engine. LayerNorm stats use ones-vector PE matmuls for the
# cross-partition sums; the per-row coefficients are broadcast back across
# partitions with K=1 matmuls. Attention runs in fp16 (values are tiny:
# random init weights scaled 0.02 + LayerNorm everywhere), with softmax
# normalization folded in as a broadcasted reciprocal multiply.
from contextlib import ExitStack

import numpy as np

import concourse.bass as bass
import concourse.mybir as mybir
import concourse.tile as tile
from concourse import bacc, bass_utils
from concourse.masks import make_identity

F32 = mybir.dt.float32
F32R = mybir.dt.float32r
F16 = mybir.dt.float16
AF = mybir.ActivationFunctionType
OP = mybir.AluOpType

D = 512
H = 8
DK = 64
S = 150
DFF = 2048
LN_EPS = 1e-5
B = 256
NCORES = 8
BB = B // NCORES          # samples per core
DC = D // 128             # 4 hidden-dim chunks
FC = DFF // 128           # 16 ffn-dim chunks
G = 4                     # samples per attention group
NBLK = 300                # row-block for full-width matmuls
GBLK = 300                # row-block inside an attention group (G*S/2)


# ----------------------------------------------------------------------------
# host-side param packing
# ----------------------------------------------------------------------------
class _Packer:
    def __init__(self, dtype):
        self.parts = []
        self.off = 0
        self.dtype = dtype

    def add(self, arr):
        a = np.ascontiguousarray(np.asarray(arr), dtype=self.dtype)
        off = self.off
        self.parts.append(a.reshape(-1))
        self.off += a.size
        return off

    def blob(self):
        if not self.parts:
            return np.zeros(1, self.dtype)
        return np.concatenate(self.parts)


def _pack_params(params):
    """Flatten params into fp32 + fp16 blobs and an offset table.

    Conventions: linear w (din,dout) row-major (k-chunk c = rows [128c,128c+128));
    bias (dout,) contiguous, loaded on-chip as [128, dout/128].
    V-projection biases of enc/dec-cross attention are folded into the output
    projection bias (softmax rows sum to 1, so  sum_j a_ij (xWV + bv)_j @ WO
    == (sum_j a_ij (xWV)_j) @ WO + bv @ WO  exactly).
    The attention output projections (enc WO, dec cross WO) go to the fp16 blob.
    """
    p32 = _Packer(np.float32)
    p16 = _Packer(np.float16)
    o = {}

    def lin(pref, lp):
        o[pref + ".w"] = p32.add(lp["w"])
        o[pref + ".b"] = p32.add(lp["b"])

    def lnp(pref, lp):
        o[pref + ".g"] = p32.add(lp["g"])
        o[pref + ".b"] = p32.add(lp["b"])

    def fcres(pref, fp):
        lin(pref + ".fi", fp["fi"])
        lin(pref + ".ff", fp["ff"])
        lnp(pref + ".ln", fp["ln"])

    def attn_out(pref, ap):
        wo = np.asarray(ap["WO"]["w"], np.float64)
        bo = np.asarray(ap["WO"]["b"], np.float64)
        bv = np.asarray(ap["WV"]["b"], np.float64)
        o[pref + ".WO.w16"] = p16.add(np.asarray(ap["WO"]["w"], np.float32))
        o[pref + ".WO.b"] = p32.add((bo + bv @ wo).astype(np.float32))

    lin("bar.init", params["bar"]["init"])
    o["bar.init.w16"] = p16.add(np.asarray(params["bar"]["init"]["w"], np.float32))
    for i, fp in enumerate(params["bar"]["res"]):
        fcres(f"bar.res{i}", fp)
    o["posT"] = p32.add(np.asarray(params["pos"], np.float32).T)  # (512,150)
    for i, ep in enumerate(params["enc"]):
        a = ep["attn"]
        lin(f"enc{i}.WQ", a["WQ"])
        lin(f"enc{i}.WK", a["WK"])
        o[f"enc{i}.WV.w"] = p32.add(a["WV"]["w"])
        attn_out(f"enc{i}", a)
        lnp(f"enc{i}.ln", ep["ln"])
        fcres(f"enc{i}.fcr", ep["fcr"])
    for i, dp in enumerate(params["dec"]):
        lin(f"dec{i}.sWV", dp["self"]["WV"])
        lin(f"dec{i}.sWO", dp["self"]["WO"])
        c = dp["cross"]
        lin(f"dec{i}.cWQ", c["WQ"])
        lin(f"dec{i}.cWK", c["WK"])
        o[f"dec{i}.cWV.w"] = p32.add(c["WV"]["w"])
        attn_out(f"dec{i}.c", c)
        lnp(f"dec{i}.ln1", dp["ln1"])
        lnp(f"dec{i}.ln2", dp["ln2"])
        fcres(f"dec{i}.fcr", dp["fcr"])
    for i, fp in enumerate(params["fcres"]):
        fcres(f"fin{i}", fp)
    lin("disc", params["fc_disc"])
    return p32.blob(), p16.blob(), o


# ----------------------------------------------------------------------------
# device-side builder
# ----------------------------------------------------------------------------
class _Builder:
    def __init__(self, ctx, tc, wf32, wf16, off, bb):
        self.nc = tc.nc
        self.tc = tc
        self.wf32 = wf32
        self.wf16 = wf16
        self.off = off
        self.bb = bb
        self.r = bb * S
        self.ng = bb // G
        nc = self.nc
        self.res = ctx.enter_context(tc.tile_pool(name="res", bufs=1))
        self.wa = ctx.enter_context(tc.tile_pool(name="wa", bufs=1))
        self.wb = ctx.enter_context(tc.tile_pool(name="wb", bufs=1))
        self.wh = ctx.enter_context(tc.tile_pool(name="wh", bufs=1))
        self.sc = ctx.enter_context(tc.tile_pool(name="sc", bufs=1))
        self.vp = ctx.enter_context(tc.tile_pool(name="vp", bufs=2))
        self.sm = ctx.enter_context(tc.tile_pool(name="sm", bufs=2))
        self.ex = ctx.enter_context(tc.tile_pool(name="ex", bufs=3))
        self.ps = ctx.enter_context(tc.tile_pool(name="ps", bufs=2, space="PSUM"))
        ctx.enter_context(nc.allow_low_precision("fp16 attention path"))

        self.xT = [self.res.tile([128, self.r], F32R, tag=f"xT{c}", name=f"xT{c}")
                   for c in range(DC)]
        self.oT = [self.res.tile([128, bb], F32R, tag=f"oT{c}", name=f"oT{c}")
                   for c in range(DC)]
        self.id16 = self.res.tile([128, 128], F16, tag="id16", name="id16")
        make_identity(nc, self.id16[:])
        self.ones16 = self.res.tile([128, 1], F16, tag="ones16", name="ones16")
        nc.vector.memset(self.ones16[:], 1.0)
        self.onesr16 = self.res.tile([1, 64], F16, tag="onesr16", name="onesr16")
        nc.vector.memset(self.onesr16[:], 1.0)
        # f32r tiles cannot be memset directly; round through a f32 copy
        self.ones32 = self.res.tile([128, 1], F32R, tag="ones32", name="ones32")
        o32a = self.res.tile([128, 1], F32, tag="o32a", name="o32a")
        nc.vector.memset(o32a[:], 1.0)
        nc.vector.tensor_copy(self.ones32[:], o32a[:])
        self.onesr32 = self.res.tile([1, 128], F32R, tag="onesr32", name="onesr32")
        o32b = self.res.tile([1, 128], F32, tag="o32b", name="o32b")
        nc.vector.memset(o32b[:], 1.0)
        nc.vector.tensor_copy(self.onesr32[:], o32b[:])
        self.epsln = self.res.tile([128, 1], F32, tag="epsln", name="epsln")
        nc.vector.memset(self.epsln[:], LN_EPS)
        self.eps9 = self.res.tile([128, 1], F32, tag="eps9", name="eps9")
        nc.vector.memset(self.eps9[:], 1e-9)

    # ---------------- loads ----------------
    def load_w(self, key, nk, dout, pool, tags):
        off = self.off[key + ".w"]
        ts = []
        for k in range(nk):
            t = pool.tile([128, dout], F32R, tag=tags[k], name=tags[k])
            self.nc.sync.dma_start(
                t[:],
                self.wf32[off + k * 128 * dout: off + (k + 1) * 128 * dout]
                .rearrange("(p n) -> p n", n=dout).bitcast(F32R))
            ts.append(t)
        return ts

    def load_w16(self, key):
        off = self.off[key + ".WO.w16"]
        ts = []
        for k in range(DC):
            t = self.wh.tile([128, D], F16, tag=f"wh{k}", name=f"wh{k}")
            self.nc.sync.dma_start(
                t[:],
                self.wf16[off + k * 128 * D: off + (k + 1) * 128 * D]
                .rearrange("(p n) -> p n", n=D))
            ts.append(t)
        return ts

    def load_vec(self, off, n, tag):
        """(n,) fp32 vector -> [128, n/128] tile (col c = chunk c)."""
        ncol = n // 128
        t = self.sm.tile([128, ncol], F32, tag=tag, name=tag)
        self.nc.sync.dma_start(
            t[:], self.wf32[off: off + n].rearrange("(c p) -> p c", p=128))
        return t

    def load_bias(self, key, n, tag):
        return self.load_vec(self.off[key + ".b"], n, tag)

    # ---------------- elementwise helpers ----------------
    def evict(self, dst, ps, bias_ap, lrelu=False):
        """PSUM -> dst with bias add; optional leaky-relu(0.01)."""
        nc = self.nc
        if bias_ap is None:
            nc.scalar.copy(dst, ps)
        else:
            nc.scalar.activation(dst, ps, AF.Identity, bias=bias_ap, scale=1.0)
        if lrelu:
            # lrelu(y) = max(0.01*y, y), one DVE op
            nc.vector.scalar_tensor_tensor(
                dst, dst, 0.01, dst, op0=OP.mult, op1=OP.max)

    # ---------------- transposed linear (scheme b) ----------------
    def linT(self, dsts, srcs, wts, bt, cols, src0=0, dst0=0, nblk=None,
             lrelu=False, wt16=False, ptag="pa"):
        """dsts[m][:, dst0+n] = act(sum_k wts[k][:,m*128:+128].T @ srcs[k][:, src0+n] + b)."""
        nc = self.nc
        nk, nm = len(wts), len(dsts)
        nblk = nblk or cols
        for m in range(nm):
            for n0 in range(0, cols, nblk):
                nb = min(nblk, cols - n0)
                ps = self.ps.tile([128, nb], F32, tag=ptag, name=ptag)
                for k in range(nk):
                    lhs = wts[k][:, m * 128:(m + 1) * 128]
                    rhs = srcs[k][:, src0 + n0: src0 + n0 + nb]
                    if not wt16:
                        lhs, rhs = (lhs), (rhs)
                    nc.tensor.matmul(ps[:], lhs, rhs,
                                     start=(k == 0), stop=(k == nk - 1))
                self.evict(dsts[m][:, dst0 + n0: dst0 + n0 + nb], ps[:],
                           None if bt is None else bt[:, m:m + 1], lrelu=lrelu)

    # ---------------- layernorm over partitions (512 = 4 chunks) ----------
    def ln_T(self, xs, pref, cols, nblk):
        """In-place layernorm of xs (transposed chunks) over the hidden dim.

        Per block: ones-matmul column sums/sumsq -> [1, nb] psum -> sbuf row;
        K=1 ones-matmul broadcasts the rows to [128, nb]; coefficients are
        then computed redundantly per partition and applied elementwise."""
        nc = self.nc
        g = self.load_vec(self.off[pref + ".g"], D, "lng")
        b = self.load_vec(self.off[pref + ".b"], D, "lnb")
        for n0 in range(0, cols, nblk):
            nb = min(nblk, cols - n0)
            sp = self.ps.tile([1, nb], F32, tag="pc", name="pc")
            for c in range(DC):
                nc.tensor.matmul(sp[:], (self.ones32[:]),
                                 (xs[c][:, n0:n0 + nb]),
                                 start=(c == 0), stop=(c == DC - 1))
            qp = self.ps.tile([1, nb], F32, tag="pd", name="pd")
            for c in range(DC):
                sq = self.sm.tile([128, nb], F32R, tag="fo", name="sqt")
                nc.vector.tensor_mul(sq[:], xs[c][:, n0:n0 + nb],
                                     xs[c][:, n0:n0 + nb])
                nc.tensor.matmul(qp[:], (self.ones32[:]), (sq[:]),
                                 start=(c == 0), stop=(c == DC - 1))
            srow = self.sm.tile([1, nb], F32R, tag="lr0", name="lr0")
            nc.vector.tensor_copy(srow[:], sp[:])
            qrow = self.sm.tile([1, nb], F32R, tag="lr1", name="lr1")
            nc.vector.tensor_copy(qrow[:], qp[:])
            bs = self.ps.tile([128, nb], F32, tag="pc", name="pcB")
            nc.tensor.matmul(bs[:], (self.onesr32[:]), (srow[:]),
                             start=True, stop=True)
            bq = self.ps.tile([128, nb], F32, tag="pd", name="pdB")
            nc.tensor.matmul(bq[:], (self.onesr32[:]), (qrow[:]),
                             start=True, stop=True)
            mean = self.sm.tile([128, nb], F32, tag="lm", name="lm", bufs=1)
            nc.vector.tensor_scalar_mul(mean[:], bs[:], 1.0 / D)
            var = self.sm.tile([128, nb], F32, tag="fo", name="var")
            nc.vector.tensor_mul(var[:], mean[:], mean[:])
            nc.vector.scalar_tensor_tensor(var[:], bq[:], 1.0 / D, var[:],
                                           op0=OP.mult, op1=OP.subtract)
            rstd = self.sm.tile([128, nb], F32, tag="lrs", name="lrs", bufs=1)
            nc.scalar.activation(rstd[:], var[:], AF.Sqrt,
                                 bias=self.epsln[:, 0:1], scale=1.0)
            nc.vector.reciprocal(rstd[:], rstd[:])
            mrs = self.sm.tile([128, nb], F32, tag="lms", name="lms", bufs=1)
            nc.vector.scalar_tensor_tensor(mrs[:], mean[:], -1.0, rstd[:],
                                           op0=OP.mult, op1=OP.mult)
            for c in range(DC):
                t1 = self.sm.tile([128, nb], F32, tag="fo", name="lnt")
                nc.vector.tensor_mul(t1[:], xs[c][:, n0:n0 + nb], rstd[:])
                nc.vector.tensor_add(t1[:], t1[:], mrs[:])
                nc.vector.tensor_scalar(xs[c][:, n0:n0 + nb], t1[:],
                                        g[:, c:c + 1], b[:, c:c + 1],
                                        op0=OP.mult, op1=OP.add)

    # ---------------- fcres block ----------------
    def fcres_T(self, xs, pref, cols, nblk):
        nc = self.nc
        w1 = self.load_w(pref + ".fi", DC, DFF, self.wa,
                         [f"wa{k}" for k in range(DC)])
        b1 = self.load_bias(pref + ".fi", DFF, "b1")
        w2 = self.load_w(pref + ".ff", FC, D, self.wb,
                         [f"wb{k}" for k in range(FC)])
        b2 = self.load_bias(pref + ".ff", D, "b2")
        for n0 in range(0, cols, nblk):
            nb = min(nblk, cols - n0)
            hs = [self.sc.tile([128, nb], F32R, tag=f"s{m}", name=f"h{m}")
                  for m in range(FC)]
            for m in range(FC):
                ps = self.ps.tile([128, nb], F32, tag="pa", name="pa")
                for k in range(DC):
                    nc.tensor.matmul(ps[:], (w1[k][:, m * 128:(m + 1) * 128]),
                                     (xs[k][:, n0:n0 + nb]),
                                     start=(k == 0), stop=(k == DC - 1))
                self.evict(hs[m][:], ps[:], b1[:, m:m + 1], lrelu=True)
            for m in range(DC):
                ps = self.ps.tile([128, nb], F32, tag="pb", name="pb")
                for k in range(FC):
                    nc.tensor.matmul(ps[:], (w2[k][:, m * 128:(m + 1) * 128]),
                                     (hs[k][:]),
                                     start=(k == 0), stop=(k == FC - 1))
                t = self.sm.tile([128, nb], F32, tag="fo", name="fo")
                self.evict(t[:], ps[:], b2[:, m:m + 1], lrelu=True)
                nc.vector.tensor_add(xs[m][:, n0:n0 + nb], t[:],
                                     xs[m][:, n0:n0 + nb])
        self.ln_T(xs, pref + ".ln", cols, nblk)

    # ---------------- preprocessing + bar embedder ----------------
    def preprocess_and_bar(self, mv_ap, qy_ap):
        nc, bb = self.nc, self.bb
        with self.tc.tile_pool(name="pp", bufs=1) as pp:
            self._preprocess_inner(pp, mv_ap, qy_ap)
        for i in range(2):
            self.fcres_T(self.xT, f"bar.res{i}", self.r, NBLK)
        # positional encoding
        po = self.off["posT"]
        for c in range(DC):
            pt = self.sm.tile([128, S], F32R, tag="pos", name="pos", bufs=1)
            nc.sync.dma_start(
                pt[:], self.wf32[po + c * 128 * S: po + (c + 1) * 128 * S]
                .rearrange("(p n) -> p n", n=S).bitcast(F32R))
            for s in range(bb):
                nc.vector.tensor_add(self.xT[c][:, s * S:(s + 1) * S],
                                     self.xT[c][:, s * S:(s + 1) * S], pt[:])

    def _preprocess_inner(self, pp, mv_ap, qy_ap):
        nc, bb = self.nc, self.bb
        inp = pp.tile([bb, 600], F32, tag="inp", name="inp")
        nc.sync.dma_start(inp[:, 0:480], mv_ap.rearrange("b t f -> b (t f)"))
        nc.sync.dma_start(inp[:, 480:600], qy_ap.rearrange("b t f -> b (t f)"))
        st6 = pp.tile([bb, 2, 6], F32, tag="st6", name="st6")
        iv = inp.rearrange("b (g n) -> b g n", g=2)
        for g2 in range(2):
            nc.vector.bn_stats(st6[:, g2, :], iv[:, g2, :])
        mv2 = pp.tile([bb, 2], F32, tag="mv2", name="mv2")
        nc.vector.bn_aggr(mv2[:], st6[:])
        sd = pp.tile([bb, 1], F32, tag="sd", name="sd")
        # torch-style unbiased std + 1e-9
        nc.scalar.activation(sd[:], mv2[:, 1:2], AF.Sqrt, bias=0.0,
                             scale=600.0 / 599.0)
        nc.scalar.activation(sd[:], sd[:], AF.Identity,
                             bias=self.eps9[0:bb, 0:1], scale=1.0)
        rec = pp.tile([bb, 1], F32, tag="rcp", name="rcp")
        nc.vector.reciprocal(rec[:], sd[:])
        nms = pp.tile([bb, 1], F32, tag="nms", name="nms")
        nc.vector.tensor_scalar(nms[:], mv2[:, 0:1], rec[0:bb, 0:1], -1.0,
                                op0=OP.mult, op1=OP.mult)
        # normalize straight into feature-major x5 [bb, 5, 150] (feat 0 = shifted)
        x5 = pp.tile([bb, 5, S], F16, tag="x5", name="x5")
        nc.vector.tensor_scalar(x5[:, 1:5, :].rearrange("b f t -> b t f"),
                                inp.rearrange("b (t f) -> b t f", f=4),
                                rec[0:bb, 0:1], nms[0:bb, 0:1],
                                op0=OP.mult, op1=OP.add)
        nc.vector.tensor_copy(x5[:, 0, 1:S], x5[:, 4, 0:S - 1])
        nc.vector.tensor_copy(x5[:, 0, 0:1], x5[:, 1, 0:1])
        # transpose to x5T [5, (s t)]: one contiguous DMA per feature row
        x5T = pp.tile([5, self.r], F16, tag="x5T", name="x5T")
        for f in range(5):
            dst = x5T[f:f + 1, :].rearrange("p (s t) -> p s t", t=S)
            nc.sync.dma_start(dst, x5[:, f, :])
        # bar init: k=5 matmul into xT
        wi = pp.tile([5, D], F16, tag="wi", name="wi")
        o = self.off["bar.init.w16"]
        nc.sync.dma_start(wi[:], self.wf16[o: o + 5 * D]
                          .rearrange("(p n) -> p n", n=D))
        bi = self.load_bias("bar.init", D, "bi")
        self.linT(self.xT, [x5T], [wi], bi, cols=self.r, nblk=NBLK, wt16=True)

    # ---------------- encoder layer ----------------
    def encoder_layer(self, i):
        nc = self.nc
        wq = self.load_w(f"enc{i}.WQ", DC, D, self.wb, [f"wb{k}" for k in range(4)])
        bq = self.load_bias(f"enc{i}.WQ", D, "bq")
        wk = self.load_w(f"enc{i}.WK", DC, D, self.wb, [f"wb{k}" for k in range(4, 8)])
        bk = self.load_bias(f"enc{i}.WK", D, "bk")
        wv = self.load_w(f"enc{i}.WV", DC, D, self.wb, [f"wb{k}" for k in range(8, 12)])
        wo = self.load_w16(f"enc{i}")
        bo = self.load_bias(f"enc{i}.WO", D, "bo")
        for g in range(self.ng):
            rb = g * G * S
            qT = [self.sc.tile([128, G * S], F16, tag=f"s{c}", name=f"qT{c}")
                  for c in range(DC)]
            kT = [self.sc.tile([128, G * S], F16, tag=f"s{4 + c}", name=f"kT{c}")
                  for c in range(DC)]
            aT = [self.sc.tile([128, G * S], F16, tag=f"s{8 + c}", name=f"aT{c}")
                  for c in range(DC)]
            self.linT(qT, self.xT, wq, bq, cols=G * S, src0=rb, nblk=GBLK)
            self.linT(kT, self.xT, wk, bk, cols=G * S, src0=rb, nblk=GBLK)
            for sl in range(G):
                s0 = rb + sl * S
                sc0 = sl * S
                v0 = self.vp.tile([128, D], F16, tag="va", name="v0")
                v1 = self.vp.tile([128, D], F16, tag="vb", name="v1")
                for vt, off, ln_ in ((v0, 0, 128), (v1, 128, 22)):
                    ps = self.ps.tile([128, D], F32, tag="pa", name="pa")
                    for k in range(DC):
                        nc.tensor.matmul(ps[0:ln_, :],
                                         (self.xT[k][:, s0 + off: s0 + off + ln_]),
                                         (wv[k][:]),
                                         start=(k == 0), stop=(k == DC - 1))
                    nc.scalar.copy(vt[0:ln_, :], ps[0:ln_, :])
                for h in range(H):
                    c, ho = h // 2, (h % 2) * 64
                    kk = kT[c][ho:ho + 64, sc0:sc0 + S]
                    qq = qT[c][ho:ho + 64, sc0:sc0 + S]
                    psA = self.ps.tile([128, S], F32, tag="pa", name="pa")
                    nc.tensor.matmul(psA[:], kk[:, 0:128], qq, start=True, stop=True)
                    psB = self.ps.tile([128, S], F32, tag="pb", name="pb")
                    nc.tensor.matmul(psB[0:22, :], kk[:, 128:S], qq,
                                     start=True, stop=True)
                    e1 = self.ex.tile([128, S], F16, tag="e1", name="e1")
                    nc.scalar.activation(e1[:], psA[:], AF.Exp, scale=0.125)
                    e2 = self.ex.tile([128, S], F16, tag="e2", name="e2")
                    nc.scalar.activation(e2[0:22, :], psB[0:22, :], AF.Exp,
                                         scale=0.125)
                    psS = self.ps.tile([1, S], F32, tag="pc", name="pc")
                    nc.tensor.matmul(psS[:], self.ones16[:], e1[:], start=True,
                                     stop=False)
                    nc.tensor.matmul(psS[:], self.ones16[0:22, :], e2[0:22, :],
                                     start=False, stop=True)
                    rec = self.ex.tile([1, S], F16, tag="rc", name="rc")
                    nc.vector.reciprocal(rec[:], psS[:])
                    psR = self.ps.tile([64, S], F32, tag="pc", name="pcR")
                    nc.tensor.matmul(psR[:], self.onesr16[:], rec[:],
                                     start=True, stop=True)
                    psV = self.ps.tile([64, S], F32, tag="pd", name="pd")
                    nc.tensor.matmul(psV[:], v0[:, h * 64:h * 64 + 64], e1[:],
                                     start=True, stop=False)
                    nc.tensor.matmul(psV[:], v1[0:22, h * 64:h * 64 + 64],
                                     e2[0:22, :], start=False, stop=True)
                    # DVE TensorTensor cannot read two PSUM operands; evict
                    # the reciprocal broadcast through ScalarE first.
                    bcR = self.ex.tile([64, S], F32, tag="bc", name="bcR")
                    nc.scalar.copy(bcR[:], psR[:])
                    nc.vector.tensor_mul(aT[c][ho:ho + 64, sc0:sc0 + S],
                                         psV[:], bcR[:])
            # output projection (+ residual)
            for m in range(DC):
                for n0 in range(0, G * S, GBLK):
                    ps = self.ps.tile([128, GBLK], F32, tag="pb", name="pbO")
                    for k in range(DC):
                        nc.tensor.matmul(ps[:], wo[k][:, m * 128:(m + 1) * 128],
                                         aT[k][:, n0:n0 + GBLK],
                                         start=(k == 0), stop=(k == DC - 1))
                    t = self.sm.tile([128, GBLK], F32, tag="fo", name="fo")
                    self.evict(t[:], ps[:], bo[:, m:m + 1])
                    nc.vector.tensor_add(self.xT[m][:, rb + n0: rb + n0 + GBLK],
                                         t[:], self.xT[m][:, rb + n0: rb + n0 + GBLK])
        self.ln_T(self.xT, f"enc{i}.ln", self.r, NBLK)
        self.fcres_T(self.xT, f"enc{i}.fcr", self.r, NBLK)

    # ---------------- decoder layer ----------------
    def decoder_layer(self, i):
        nc, bb = self.nc, self.bb
        # self-attention == WO(WV out + bv) + bo (softmax over a single key)
        wvs = self.load_w(f"dec{i}.sWV", DC, D, self.wb, [f"wb{k}" for k in range(4)])
        bvs = self.load_bias(f"dec{i}.sWV", D, "bq")
        wos = self.load_w(f"dec{i}.sWO", DC, D, self.wb, [f"wb{k}" for k in range(4, 8)])
        bos = self.load_bias(f"dec{i}.sWO", D, "bk")
        t1 = [self.sm.tile([128, bb], F32R, tag=f"d{c}", name=f"d{c}")
              for c in range(DC)]
        self.linT(t1, self.oT, wvs, bvs, cols=bb, ptag="pa")
        for m in range(DC):
            ps = self.ps.tile([128, bb], F32, tag="pb", name="pb")
            for k in range(DC):
                nc.tensor.matmul(ps[:], (wos[k][:, m * 128:(m + 1) * 128]),
                                 (t1[k][:]), start=(k == 0), stop=(k == DC - 1))
            t = self.sm.tile([128, bb], F32, tag="fo", name="fo")
            self.evict(t[:], ps[:], bos[:, m:m + 1])
            nc.vector.tensor_add(self.oT[m][:], t[:], self.oT[m][:])
        self.ln_T(self.oT, f"dec{i}.ln1", bb, bb)

        # cross attention
        wqc = self.load_w(f"dec{i}.cWQ", DC, D, self.wb, [f"wb{k}" for k in range(8, 12)])
        bqc = self.load_bias(f"dec{i}.cWQ", D, "bq")
        qT16 = [self.sm.tile([128, bb], F16, tag=f"q{c}", name=f"q{c}")
                for c in range(DC)]
        self.linT(qT16, self.oT, wqc, bqc, cols=bb, ptag="pa")
        # block-diag q: bd[c][p, s, h] = q_s[c*128+p] iff h == 2c + p//64
        bd = [self.sm.tile([128, bb, 8], F16, tag=f"bd{c}", name=f"bd{c}",
                           bufs=1) for c in range(DC)]
        for c in range(DC):
            nc.vector.memset(bd[c][:], 0.0)
            nc.sync.dma_start(bd[c][0:64, :, 2 * c], qT16[c][0:64, :])
            nc.sync.dma_start(bd[c][64:128, :, 2 * c + 1], qT16[c][64:128, :])
        wkc = self.load_w(f"dec{i}.cWK", DC, D, self.wb, [f"wb{k}" for k in range(12, 16)])
        bkc = self.load_bias(f"dec{i}.cWK", D, "bk")
        wvc = self.load_w(f"dec{i}.cWV", DC, D, self.wb, [f"wb{k}" for k in range(4)])
        woc = self.load_w16(f"dec{i}.c")
        boc = self.load_bias(f"dec{i}.c.WO", D, "bo")
        stag = [self.sc.tile([128, bb, 8], F16, tag=f"s{12 + dd}", name=f"stag{dd}")
                for dd in range(DC)]
        for g in range(self.ng):
            rb = g * G * S
            kTd = [self.sc.tile([128, G * S], F16, tag=f"s{4 + c}", name=f"kTd{c}")
                   for c in range(DC)]
            self.linT(kTd, self.xT, wkc, bkc, cols=G * S, src0=rb, nblk=GBLK,
                      ptag="pa")
            for sl in range(G):
                sg = g * G + sl
                s0 = rb + sl * S
                v0 = self.vp.tile([128, D], F16, tag="va", name="v0")
                v1 = self.vp.tile([128, D], F16, tag="vb", name="v1")
                for vt, off, ln_ in ((v0, 0, 128), (v1, 128, 22)):
                    ps = self.ps.tile([128, D], F32, tag="pa", name="pa")
                    for k in range(DC):
                        nc.tensor.matmul(ps[0:ln_, :],
                                         (self.xT[k][:, s0 + off: s0 + off + ln_]),
                                         (wvc[k][:]),
                                         start=(k == 0), stop=(k == DC - 1))
                    nc.scalar.copy(vt[0:ln_, :], ps[0:ln_, :])
                psc = self.ps.tile([8, S], F32, tag="pc", name="pc")
                for c in range(DC):
                    nc.tensor.matmul(psc[:], bd[c][:, sg, :],
                                     kTd[c][:, sl * S:(sl + 1) * S],
                                     start=(c == 0), stop=(c == DC - 1))
                es = self.ex.tile([8, S], F32, tag="e1", name="es")
                sums8 = self.ex.tile([8, 1], F32, tag="rc", name="sums8")
                nc.scalar.activation(es[:], psc[:], AF.Exp, scale=0.125,
                                     accum_out=sums8[:])
                rec8 = self.ex.tile([8, 1], F32, tag="r8", name="rec8")
                nc.vector.reciprocal(rec8[:], sums8[:])
                a16 = self.ex.tile([8, S], F16, tag="e2", name="a16")
                nc.vector.tensor_scalar_mul(a16[:], es[:], rec8[0:8, 0:1])
                pt1 = self.ps.tile([128, 8], F16, tag="pb", name="pb")
                nc.tensor.transpose(pt1[:], a16[:, 0:128], self.id16[0:8, 0:8])
                aT1 = self.ex.tile([128, 8], F16, tag="a1", name="aT1")
                nc.scalar.copy(aT1[:], pt1[:])
                pt2 = self.ps.tile([128, 8], F16, tag="pb", name="pb2")
                nc.tensor.transpose(pt2[0:22, :], a16[:, 128:S],
                                    self.id16[0:8, 0:8])
                aT2 = self.ex.tile([128, 8], F16, tag="a2", name="aT2")
                nc.scalar.copy(aT2[0:22, :], pt2[0:22, :])
                for dd in range(DC):
                    pv = self.ps.tile([128, 8], F32, tag="pd", name="pd")
                    nc.tensor.matmul(pv[:], v0[:, dd * 128:(dd + 1) * 128],
                                     aT1[:], start=True, stop=False)
                    nc.tensor.matmul(pv[:], v1[0:22, dd * 128:(dd + 1) * 128],
                                     aT2[0:22, :], start=False, stop=True)
                    nc.scalar.copy(stag[dd][:, sg, :], pv[:])
        # diagonal head extraction + output projection
        aoT = [self.sm.tile([128, bb], F16, tag=f"ao{c}", name=f"ao{c}")
               for c in range(DC)]
        for dd in range(DC):
            for hf in range(2):
                nc.sync.dma_start(aoT[dd][64 * hf:64 * hf + 64, :],
                                  stag[dd][64 * hf:64 * hf + 64, :, 2 * dd + hf])
        for m in range(DC):
            ps = self.ps.tile([128, bb], F32, tag="pb", name="pbO")
            for k in range(DC):
                nc.tensor.matmul(ps[:], woc[k][:, m * 128:(m + 1) * 128],
                                 aoT[k][:], start=(k == 0), stop=(k == DC - 1))
            t = self.sm.tile([128, bb], F32, tag="fo", name="fo")
            self.evict(t[:], ps[:], boc[:, m:m + 1])
            nc.vector.tensor_add(self.oT[m][:], t[:], self.oT[m][:])
        self.ln_T(self.oT, f"dec{i}.ln2", bb, bb)
        self.fcres_T(self.oT, f"dec{i}.fcr", bb, bb)

    # ---------------- final head ----------------
    def final(self, out_ap):
        nc, bb = self.nc, self.bb
        for i in range(4):
            self.fcres_T(self.oT, f"fin{i}", bb, bb)
        wd = self.sm.tile([128, DC], F32R, tag="wd", name="wd")
        odw = self.off["disc.w"]
        nc.sync.dma_start(
            wd[:], self.wf32[odw: odw + D]
            .rearrange("(c p) -> p c", p=128).bitcast(F32R))
        ob = self.off["disc.b"]
        bd_ = self.sm.tile([1, 1], F32, tag="bd_", name="bd_")
        nc.sync.dma_start(bd_[:], self.wf32[ob:ob + 1].rearrange("(p n) -> p n", n=1))
        psd = self.ps.tile([1, bb], F32, tag="pa", name="pa")
        for c in range(DC):
            nc.tensor.matmul(psd[:], (wd[:, c:c + 1]), (self.oT[c][:]),
                             start=(c == 0), stop=(c == DC - 1))
        sig = self.sm.tile([1, bb], F32, tag="sig", name="sig")
        nc.scalar.activation(sig[:], psd[:], AF.Sigmoid, bias=bd_[0:1, 0:1],
                             scale=1.0)
        nc.sync.dma_start(out_ap.rearrange("b one -> one b"), sig[:])

    def build(self, mv_ap, qy_ap, out_ap):
        nc = self.nc
        self.preprocess_and_bar(mv_ap, qy_ap)
        for i in range(4):
            self.encoder_layer(i)
        z0 = self.sm.tile([128, self.bb], F32, tag="fo", name="z0")
        nc.vector.memset(z0[:], 0.0)
        for c in range(DC):
            nc.vector.tensor_copy(self.oT[c][:], z0[:])
        for i in range(4):
            self.decoder_layer(i)
        self.final(out_ap)


def build_module(bb, n32, n16):
    """Build the Bass module for bb samples/core. Returns (nc, names)."""
    nc = bacc.Bacc("TRN2", target_bir_lowering=False, debug=False,
                   enable_asserts=False, num_devices=NCORES)
    mv = nc.dram_tensor("mv", (bb, 120, 4), F32, kind="ExternalInput").ap()
    qy = nc.dram_tensor("qy", (bb, 30, 4), F32, kind="ExternalInput").ap()
    w32 = nc.dram_tensor("w32", (n32,), F32, kind="ExternalInput").ap()
    w16 = nc.dram_tensor("w16", (n16,), F16, kind="ExternalInput").ap()
    out = nc.dram_tensor("out", (bb, 1), F32, kind="ExternalOutput").ap()
    return nc, mv, qy, w32, w16, out


_CACHE = {}


def _trace(bb, n32, n16, off):
    key = (bb, n32, n16)
    if key in _CACHE:
        return _CACHE[key]
    nc, mv, qy, w32, w16, out = build_module(bb, n32, n16)
    with tile.TileContext(nc) as tc:
        with ExitStack() as ctx:
            b = _Builder(ctx, tc, w32, w16, off, bb)
            b.build(mv, qy, out)
    nc.compile()
    _CACHE[key] = nc
    return nc


def kernel(market_values, query, params):
    w32, w16, off = _pack_params(params)
    mv = np.ascontiguousarray(market_values, np.float32)
    qy = np.ascontiguousarray(query, np.float32)
    nc = _trace(BB, w32.size, w16.size, off)
    in_maps = []
    for c in range(NCORES):
        in_maps.append({
            "mv": mv[c * BB:(c + 1) * BB],
            "qy": qy[c * BB:(c + 1) * BB],
            "w32": w32,
            "w16": w16,
        })
    res = bass_utils.run_bass_kernel_spmd(nc, in_maps, core_ids=list(range(NCORES)))
    return np.concatenate([res.results[c]["out"] for c in range(NCORES)], axis=0)
